# revision 31
# baseline (speedup 1.0000x reference)
"""DisMaxLossFirstPart forward on 8 Trainium2 NeuronCores.

logits = -(iso + mean_c(iso)) / temperature
  iso   = |distance_scale| * sqrt(max(2 - 2*cos(f_b, p_c), 0)) / sqrt(2)
        = sqrt(ds^2 * max(1 - cos(f_b, p_c), 0))

Data-parallel: batch (16384) sharded 8 ways across the cores; prototypes
replicated; no collectives (the per-row mean is local).

Per-core program (B_s = 2048 rows), all engines balanced:
  - prototypes: load fp32, row-normalize (ACT Square+accum -> sqrt -> DVE
    recip -> scale+cast bf16), transpose via identity-matmul on the PE ->
    pT[k] tiles [128f x 1000c].  jg-outer round order so the first 512
    columns (prototype row-tiles 0-3) are ready early - they alone feed
    the c-chunk-0 main matmuls; PSUM->SBUF copies alternate DVE/ACT.
  - per 128-row feature block (prep pipelined one block ahead of compute):
    load fp32, cast to bf16 *negated* on DVE (so the iso activation scale
    stays positive), sumsq via ACT Square+accum on the bf16 values (so the
    norm matches the matmul operand bit-for-bit), 8 PE identity-transposes
    -> fT; then per c-chunk (512 | 488 cols, each its own 1-bank PSUM
    tile) 8 accumulating bf16 matmuls and one
    iso = Sqrt(scale_b * psum + ds^2) activation with per-partition
    scale_b = +ds^2/||f_b|| (psum holds -G) and accum_out as the row-sum;
    m_b = (rs0+rs1) * (-1/T)/C on DVE; final logits = (-1/T)*iso + m_b on
    GPSIMD/DVE (alternating blocks); DMA out fp32.

PSUM banks are partitioned statically: fT transposes 2, main matmuls 4,
prototype preamble 2 - the pools are opened in that order so no phase
inherits a bank-reuse dependency on another.

distance_scale / temperature are [1]-element runtime inputs; their values
are baked into the program as immediates (the program is rebuilt per call,
which is correct for any input values at the cost of a recompile).
"""

import os

import numpy as np

N_CORES = 8
B, F, C = 16384, 1024, 1000
BS = B // N_CORES          # 2048 rows per core
NB = BS // 128             # 16 feature blocks per core
KT = F // 128              # 8 contraction chunks
CHUNKS = ((0, 512), (512, 488))   # c-chunks, aligned to prototype jg halves
PJ = (C + 127) // 128      # 8 prototype row-tiles (last one 104 rows)


def _build_program(ds2: float, neg_inv_t: float):
    from contextlib import ExitStack

    import concourse.tile as tile
    from concourse import bacc, mybir
    from concourse.masks import make_identity

    f32 = mybir.dt.float32
    bf16 = mybir.dt.bfloat16
    AF = mybir.ActivationFunctionType
    ALU = mybir.AluOpType

    inv_ds4 = (1.0 / ds2) ** 2 if ds2 != 1.0 else 1.0

    nc = bacc.Bacc("TRN2", target_bir_lowering=False, debug=False,
                   num_devices=N_CORES)

    fdr = nc.dram_tensor("features", [BS, F], f32, kind="ExternalInput").ap()
    pdr = nc.dram_tensor("prototypes", [C, F], f32, kind="ExternalInput").ap()
    odr = nc.dram_tensor("out", [BS, C], f32, kind="ExternalOutput").ap()

    with tile.TileContext(nc) as tc, ExitStack() as ctx:
        const_pool = ctx.enter_context(tc.tile_pool(name="const", bufs=1))
        ident = const_pool.tile([128, 128], bf16, tag="ident")
        make_identity(nc, ident[:])
        bias_ds2 = const_pool.tile([128, 1], f32, tag="bias_ds2")
        nc.vector.memset(bias_ds2[:], ds2)

        # persistent transposed prototypes: pT[k] is [128 (f in chunk k), C]
        pT_pool = ctx.enter_context(tc.tile_pool(name="pT", bufs=1))
        pT = [pT_pool.tile([128, C], bf16, tag=f"pT{k}", name=f"pT{k}")
              for k in range(KT)]

        # Main-loop PSUM pools are opened BEFORE the preamble's so the bank
        # ranges are disjoint (ftps 0-1, spsum 2-5, ppsum 6-7).  With stack
        # reuse instead, every early main matmul would inherit a dependency
        # on the full preamble PSUM drain.
        ftps = ctx.enter_context(tc.tile_pool(name="ftps", bufs=2, space="PSUM"))
        spsum = ctx.enter_context(tc.tile_pool(name="spsum", bufs=4, space="PSUM"))

        fload = ctx.enter_context(tc.tile_pool(name="fload", bufs=5))

        # ---- prototype preamble -------------------------------------------
        with tc.tile_pool(name="pload", bufs=1) as pload, \
             tc.tile_pool(name="pbf", bufs=1) as pbfp, \
             tc.tile_pool(name="ppsum", bufs=2, space="PSUM") as ppsum, \
             tc.tile_pool(name="psmall", bufs=1) as psmall:
            p_bf = []
            for j in range(PJ):
                rows = min(128, C - j * 128)
                praw = pload.tile([128, F], f32, tag=f"praw{j}",
                                  name=f"praw{j}")
                nc.sync.dma_start(out=praw[:rows],
                                  in_=pdr[j * 128: j * 128 + rows])
                sq = pload.tile([128, F], f32, tag="psq", bufs=1,
                                name=f"psq{j}")
                ss = psmall.tile([128, 1], f32, tag=f"pss{j}")
                nc.scalar.activation(sq[:rows], praw[:rows], AF.Square,
                                     accum_out=ss[:rows])
                nrm = psmall.tile([128, 1], f32, tag=f"pnrm{j}")
                nc.scalar.activation(nrm[:rows], ss[:rows], AF.Sqrt)
                inv = psmall.tile([128, 1], f32, tag=f"pinv{j}")
                nc.vector.reciprocal(inv[:rows], nrm[:rows])
                pb = pbfp.tile([128, F], bf16, tag=f"pbf{j}")
                nc.vector.tensor_scalar_mul(pb[:rows], praw[:rows], inv[:rows])
                p_bf.append((pb, rows))
            # transpose: jg-outer so pT[:][:, 0:512] (chunk-0 rhs) is ready
            # after only the first 4 prototype tiles.
            for jg in range(PJ // 4):
                for k in range(KT):
                    cols = sum(r for _, r in p_bf[jg * 4: jg * 4 + 4])
                    pt_ps = ppsum.tile([128, 512], f32, tag="ptps")
                    for jj in range(4):
                        pb, rows = p_bf[jg * 4 + jj]
                        nc.tensor.matmul(
                            pt_ps[:, jj * 128: jj * 128 + rows],
                            lhsT=pb[:rows, k * 128:(k + 1) * 128],
                            rhs=ident[:rows, :rows], start=True, stop=True)
                    if k % 2 == 0:
                        nc.vector.tensor_copy(
                            pT[k][:, jg * 512: jg * 512 + cols],
                            pt_ps[:, :cols])
                    else:
                        nc.scalar.copy(
                            pT[k][:, jg * 512: jg * 512 + cols],
                            pt_ps[:, :cols])

        # ---- main loop over 16 feature blocks -----------------------------
        with tc.tile_pool(name="fbf", bufs=3) as fbfp, \
             tc.tile_pool(name="fsq", bufs=1) as fsqp, \
             tc.tile_pool(name="fT", bufs=2) as fTp, \
             tc.tile_pool(name="iso", bufs=5) as isop, \
             tc.tile_pool(name="osb", bufs=5) as osbp, \
             tc.tile_pool(name="small", bufs=6) as smallp:

            def prep(bi):
                """Load + negated-cast + norm-chain + PE transpose."""
                fr = fload.tile([128, F], f32, tag="fraw")
                nc.sync.dma_start(out=fr[:], in_=fdr[bi * 128:(bi + 1) * 128])
                fb = fbfp.tile([128, F], bf16, tag="fbf")
                nc.vector.tensor_scalar_mul(fb[:], fr[:], -1.0)  # cast+negate
                sqscr = fsqp.tile([128, F], f32, tag="sqscr")
                ss = smallp.tile([128, 1], f32, tag="fss")
                nc.scalar.activation(sqscr[:], fb[:], AF.Square, accum_out=ss[:])
                # ||f||/ds^2, then scale_b = ds^2/||f|| (positive; psum = -G)
                nrm = smallp.tile([128, 1], f32, tag="fnrm")
                nc.scalar.activation(nrm[:], ss[:], AF.Sqrt, scale=inv_ds4)
                scl = smallp.tile([128, 1], f32, tag="fscl")
                nc.vector.reciprocal(scl[:], nrm[:])
                # both 4-transpose groups back-to-back on PE, then the two
                # PSUM->SBUF copies - fewer PE burst transitions
                ft_pss = []
                for g in range(2):
                    ft_ps = ftps.tile([128, 512], f32, tag="ftps")
                    for kk in range(4):
                        k = g * 4 + kk
                        nc.tensor.matmul(
                            ft_ps[:, kk * 128:(kk + 1) * 128],
                            lhsT=fb[:, k * 128:(k + 1) * 128],
                            rhs=ident[:], start=True, stop=True)
                    ft_pss.append(ft_ps)
                fT = []
                for g in range(2):
                    fts = fTp.tile([128, 512], bf16, tag=f"fT{g}")
                    nc.vector.tensor_copy(fts[:], ft_pss[g][:])
                    fT.append(fts)
                return fT, scl

            def compute(bi, st):
                fT, scl = st
                iso = isop.tile([128, C], f32, tag="iso")
                rs = []
                for ci, (cbase, cw) in enumerate(CHUNKS):
                    sp = spsum.tile([128, 512], f32, tag="spsum")
                    for k in range(KT):
                        g, kk = divmod(k, 4)
                        nc.tensor.matmul(
                            sp[:, :cw],
                            lhsT=fT[g][:, kk * 128:(kk + 1) * 128],
                            rhs=pT[k][:, cbase:cbase + cw],
                            start=(k == 0), stop=(k == KT - 1))
                    r = smallp.tile([128, 1], f32, tag=f"rs{ci}")
                    nc.scalar.activation(iso[:, cbase:cbase + cw], sp[:, :cw],
                                         AF.Sqrt, bias=bias_ds2[:],
                                         scale=scl[:], accum_out=r[:])
                    rs.append(r)
                m = smallp.tile([128, 1], f32, tag="m")
                nc.vector.tensor_scalar(m[:], rs[0][:], rs[1][:],
                                        neg_inv_t / C, ALU.add, ALU.mult)
                ob = osbp.tile([128, C], f32, tag="osb")
                eng = nc.gpsimd if bi % 2 == 0 else nc.vector
                eng.tensor_scalar(ob[:], iso[:], neg_inv_t, m[:],
                                  ALU.mult, ALU.add)
                nc.sync.dma_start(out=odr[bi * 128:(bi + 1) * 128], in_=ob[:])

            sts = [prep(0), prep(1)]
            for bi in range(NB):
                if bi + 2 < NB:
                    sts.append(prep(bi + 2))
                compute(bi, sts[bi])

    nc.compile()
    return nc


def kernel(features, prototypes, distance_scale, temperature):
    from concourse.bass_utils import run_bass_kernel_spmd

    features = np.ascontiguousarray(features, dtype=np.float32)
    prototypes = np.ascontiguousarray(prototypes, dtype=np.float32)
    ds2 = float(abs(float(np.asarray(distance_scale).reshape(-1)[0])) ** 2)
    neg_inv_t = -1.0 / float(np.asarray(temperature).reshape(-1)[0])

    nc = _build_program(ds2, neg_inv_t)

    in_maps = [{"features": features[i * BS:(i + 1) * BS],
                "prototypes": prototypes} for i in range(N_CORES)]

    trace_dir = os.environ.get("KERNEL_TRACE_DIR")
    if trace_dir:
        res = run_bass_kernel_spmd(nc, in_maps, list(range(N_CORES)),
                                   trace=True, tmpdir=trace_dir)
        print(f"HW exec time: {res.exec_time_ns} ns")
        print(f"mean core exec time: {res.mean_exec_time_ns} ns")
    else:
        res = run_bass_kernel_spmd(nc, in_maps, list(range(N_CORES)))

    return np.concatenate([res.results[i]["out"] for i in range(N_CORES)],
                          axis=0)


# revision 32
# speedup vs baseline: 1.0247x; 1.0247x over previous
"""DisMaxLossFirstPart forward on 8 Trainium2 NeuronCores.

logits = -(iso + mean_c(iso)) / temperature
  iso   = |distance_scale| * sqrt(max(2 - 2*cos(f_b, p_c), 0)) / sqrt(2)
        = sqrt(ds^2 * max(1 - cos(f_b, p_c), 0))

Data-parallel: batch (16384) sharded 8 ways across the cores; prototypes
replicated; no collectives (the per-row mean is local).

Per-core program (B_s = 2048 rows), all engines balanced:
  - prototypes: load fp32, row-normalize (ACT Square+accum -> sqrt -> DVE
    recip -> scale+cast bf16), transpose via identity-matmul on the PE ->
    pT[k] tiles [128f x 1000c].  jg-outer round order so the first 512
    columns (prototype row-tiles 0-3) are ready early - they alone feed
    the c-chunk-0 main matmuls; PSUM->SBUF copies alternate DVE/ACT.
  - per 128-row feature block (prep pipelined one block ahead of compute):
    load fp32, cast to bf16 *negated* on DVE (so the iso activation scale
    stays positive), sumsq via ACT Square+accum on the bf16 values (so the
    norm matches the matmul operand bit-for-bit), 8 PE identity-transposes
    -> fT; then per c-chunk (512 | 488 cols, each its own 1-bank PSUM
    tile) 8 accumulating bf16 matmuls and one
    iso = Sqrt(scale_b * psum + ds^2) activation with per-partition
    scale_b = +ds^2/||f_b|| (psum holds -G) and accum_out as the row-sum;
    m_b = (rs0+rs1) * (-1/T)/C on DVE; final logits = (-1/T)*iso + m_b on
    GPSIMD/DVE (alternating blocks); DMA out fp32.

PSUM banks are partitioned statically: fT transposes 2, main matmuls 4,
prototype preamble 2 - the pools are opened in that order so no phase
inherits a bank-reuse dependency on another.

distance_scale / temperature are [1]-element runtime inputs; their values
are baked into the program as immediates (the program is rebuilt per call,
which is correct for any input values at the cost of a recompile).
"""

import os

import numpy as np

N_CORES = 8
B, F, C = 16384, 1024, 1000
BS = B // N_CORES          # 2048 rows per core
NB = BS // 128             # 16 feature blocks per core
KT = F // 128              # 8 contraction chunks
CHUNKS = ((0, 512), (512, 488))   # c-chunks, aligned to prototype jg halves
PJ = (C + 127) // 128      # 8 prototype row-tiles (last one 104 rows)


def _build_program(ds2: float, neg_inv_t: float):
    from contextlib import ExitStack

    import concourse.tile as tile
    from concourse import bacc, mybir
    from concourse.masks import make_identity

    f32 = mybir.dt.float32
    bf16 = mybir.dt.bfloat16
    AF = mybir.ActivationFunctionType
    ALU = mybir.AluOpType

    inv_ds4 = (1.0 / ds2) ** 2 if ds2 != 1.0 else 1.0

    nc = bacc.Bacc("TRN2", target_bir_lowering=False, debug=False,
                   num_devices=N_CORES)

    fdr = nc.dram_tensor("features", [BS, F], f32, kind="ExternalInput").ap()
    pdr = nc.dram_tensor("prototypes", [C, F], f32, kind="ExternalInput").ap()
    odr = nc.dram_tensor("out", [BS, C], f32, kind="ExternalOutput").ap()

    with tile.TileContext(nc) as tc, ExitStack() as ctx:
        const_pool = ctx.enter_context(tc.tile_pool(name="const", bufs=1))
        ident = const_pool.tile([128, 128], bf16, tag="ident")
        make_identity(nc, ident[:])
        bias_ds2 = const_pool.tile([128, 1], f32, tag="bias_ds2")
        nc.vector.memset(bias_ds2[:], ds2)

        # persistent transposed prototypes: pT[k] is [128 (f in chunk k), C]
        pT_pool = ctx.enter_context(tc.tile_pool(name="pT", bufs=1))
        pT = [pT_pool.tile([128, C], bf16, tag=f"pT{k}", name=f"pT{k}")
              for k in range(KT)]

        # Main-loop PSUM pools are opened BEFORE the preamble's so the bank
        # ranges are disjoint (ftps 0-1, spsum 2-5, ppsum 6-7).  With stack
        # reuse instead, every early main matmul would inherit a dependency
        # on the full preamble PSUM drain.
        ftps = ctx.enter_context(tc.tile_pool(name="ftps", bufs=2, space="PSUM"))
        spsum = ctx.enter_context(tc.tile_pool(name="spsum", bufs=4, space="PSUM"))

        fload = ctx.enter_context(tc.tile_pool(name="fload", bufs=5))

        # ---- prototype preamble -------------------------------------------
        with tc.tile_pool(name="pload", bufs=1) as pload, \
             tc.tile_pool(name="pbf", bufs=1) as pbfp, \
             tc.tile_pool(name="ppsum", bufs=2, space="PSUM") as ppsum, \
             tc.tile_pool(name="psmall", bufs=1) as psmall:
            p_bf = []
            for j in range(PJ):
                rows = min(128, C - j * 128)
                praw = pload.tile([128, F], f32, tag=f"praw{j}",
                                  name=f"praw{j}")
                nc.sync.dma_start(out=praw[:rows],
                                  in_=pdr[j * 128: j * 128 + rows])
                sq = pload.tile([128, F], f32, tag="psq", bufs=1,
                                name=f"psq{j}")
                ss = psmall.tile([128, 1], f32, tag=f"pss{j}")
                nc.scalar.activation(sq[:rows], praw[:rows], AF.Square,
                                     accum_out=ss[:rows])
                nrm = psmall.tile([128, 1], f32, tag=f"pnrm{j}")
                nc.scalar.activation(nrm[:rows], ss[:rows], AF.Sqrt)
                inv = psmall.tile([128, 1], f32, tag=f"pinv{j}")
                nc.vector.reciprocal(inv[:rows], nrm[:rows])
                pb = pbfp.tile([128, F], bf16, tag=f"pbf{j}")
                nc.vector.tensor_scalar_mul(pb[:rows], praw[:rows], inv[:rows])
                p_bf.append((pb, rows))
            # transpose: jg-outer so pT[:][:, 0:512] (chunk-0 rhs) is ready
            # after only the first 4 prototype tiles.
            def p_round(jg, k, j0, nj):
                cols = sum(r for _, r in p_bf[j0: j0 + nj])
                base = (j0 - jg * 4) * 128
                pt_ps = ppsum.tile([128, 512], f32, tag="ptps",
                                   name=f"ptps_{jg}_{k}_{j0}")
                for jj in range(nj):
                    pb, rows = p_bf[j0 + jj]
                    nc.tensor.matmul(
                        pt_ps[:, jj * 128: jj * 128 + rows],
                        lhsT=pb[:rows, k * 128:(k + 1) * 128],
                        rhs=ident[:rows, :rows], start=True, stop=True)
                eng = nc.vector if k % 2 == 0 else nc.scalar
                dst = pT[k][:, jg * 512 + base: jg * 512 + base + cols]
                if k % 2 == 0:
                    nc.vector.tensor_copy(dst, pt_ps[:, :cols])
                else:
                    nc.scalar.copy(dst, pt_ps[:, :cols])

            # k=0,1 of jg0 in 2-tile packs (ready after prototype tiles 0-1),
            # everything else in 4-tile packs
            for k in (0, 1):
                p_round(0, k, 0, 2)
                p_round(0, k, 2, 2)
            for k in range(2, KT):
                p_round(0, k, 0, 4)
            for k in range(KT):
                p_round(1, k, 4, 4)

        # ---- main loop over 16 feature blocks -----------------------------
        with tc.tile_pool(name="fbf", bufs=3) as fbfp, \
             tc.tile_pool(name="fsq", bufs=1) as fsqp, \
             tc.tile_pool(name="fT", bufs=2) as fTp, \
             tc.tile_pool(name="iso", bufs=5) as isop, \
             tc.tile_pool(name="osb", bufs=5) as osbp, \
             tc.tile_pool(name="small", bufs=6) as smallp:

            def prep(bi):
                """Load + negated-cast + norm-chain + PE transpose."""
                fr = fload.tile([128, F], f32, tag="fraw")
                nc.sync.dma_start(out=fr[:], in_=fdr[bi * 128:(bi + 1) * 128])
                fb = fbfp.tile([128, F], bf16, tag="fbf")
                nc.vector.tensor_scalar_mul(fb[:], fr[:], -1.0)  # cast+negate
                sqscr = fsqp.tile([128, F], f32, tag="sqscr")
                ss = smallp.tile([128, 1], f32, tag="fss")
                nc.scalar.activation(sqscr[:], fb[:], AF.Square, accum_out=ss[:])
                # ||f||/ds^2, then scale_b = ds^2/||f|| (positive; psum = -G)
                nrm = smallp.tile([128, 1], f32, tag="fnrm")
                nc.scalar.activation(nrm[:], ss[:], AF.Sqrt, scale=inv_ds4)
                scl = smallp.tile([128, 1], f32, tag="fscl")
                nc.vector.reciprocal(scl[:], nrm[:])
                # both 4-transpose groups back-to-back on PE, then the two
                # PSUM->SBUF copies - fewer PE burst transitions
                ft_pss = []
                for g in range(2):
                    ft_ps = ftps.tile([128, 512], f32, tag="ftps")
                    for kk in range(4):
                        k = g * 4 + kk
                        nc.tensor.matmul(
                            ft_ps[:, kk * 128:(kk + 1) * 128],
                            lhsT=fb[:, k * 128:(k + 1) * 128],
                            rhs=ident[:], start=True, stop=True)
                    ft_pss.append(ft_ps)
                fT = []
                for g in range(2):
                    fts = fTp.tile([128, 512], bf16, tag=f"fT{g}")
                    nc.vector.tensor_copy(fts[:], ft_pss[g][:])
                    fT.append(fts)
                return fT, scl

            def compute(bi, st):
                fT, scl = st
                iso = isop.tile([128, C], f32, tag="iso")
                rs = []
                for ci, (cbase, cw) in enumerate(CHUNKS):
                    sp = spsum.tile([128, 512], f32, tag="spsum")
                    for k in range(KT):
                        g, kk = divmod(k, 4)
                        nc.tensor.matmul(
                            sp[:, :cw],
                            lhsT=fT[g][:, kk * 128:(kk + 1) * 128],
                            rhs=pT[k][:, cbase:cbase + cw],
                            start=(k == 0), stop=(k == KT - 1))
                    r = smallp.tile([128, 1], f32, tag=f"rs{ci}")
                    nc.scalar.activation(iso[:, cbase:cbase + cw], sp[:, :cw],
                                         AF.Sqrt, bias=bias_ds2[:],
                                         scale=scl[:], accum_out=r[:])
                    rs.append(r)
                m = smallp.tile([128, 1], f32, tag="m")
                nc.vector.tensor_scalar(m[:], rs[0][:], rs[1][:],
                                        neg_inv_t / C, ALU.add, ALU.mult)
                ob = osbp.tile([128, C], f32, tag="osb")
                eng = nc.gpsimd if bi % 2 == 0 else nc.vector
                eng.tensor_scalar(ob[:], iso[:], neg_inv_t, m[:],
                                  ALU.mult, ALU.add)
                nc.sync.dma_start(out=odr[bi * 128:(bi + 1) * 128], in_=ob[:])

            st = prep(0)
            for bi in range(NB):
                nxt = prep(bi + 1) if bi + 1 < NB else None
                compute(bi, st)
                st = nxt

    nc.compile()
    return nc


def kernel(features, prototypes, distance_scale, temperature):
    from concourse.bass_utils import run_bass_kernel_spmd

    features = np.ascontiguousarray(features, dtype=np.float32)
    prototypes = np.ascontiguousarray(prototypes, dtype=np.float32)
    ds2 = float(abs(float(np.asarray(distance_scale).reshape(-1)[0])) ** 2)
    neg_inv_t = -1.0 / float(np.asarray(temperature).reshape(-1)[0])

    nc = _build_program(ds2, neg_inv_t)

    in_maps = [{"features": features[i * BS:(i + 1) * BS],
                "prototypes": prototypes} for i in range(N_CORES)]

    trace_dir = os.environ.get("KERNEL_TRACE_DIR")
    if trace_dir:
        res = run_bass_kernel_spmd(nc, in_maps, list(range(N_CORES)),
                                   trace=True, tmpdir=trace_dir)
        print(f"HW exec time: {res.exec_time_ns} ns")
        print(f"mean core exec time: {res.mean_exec_time_ns} ns")
    else:
        res = run_bass_kernel_spmd(nc, in_maps, list(range(N_CORES)))

    return np.concatenate([res.results[i]["out"] for i in range(N_CORES)],
                          axis=0)
